# revision 1
# baseline (speedup 1.0000x reference)
"""Trainium2 Bass kernel for nn_BlockedMLP (dense_mlp, 8 cores).

Strategy:
  - 8-way data parallel over the batch (B=2048 -> 256 rows/core), weights
    replicated. No collectives.
  - The BSR fc2 (50% block density, 32x32 blocks) is scattered into a dense
    [H, H] matrix on the host: on the PE array a matmul costs N streamed
    columns regardless of contraction K, so 32x32 sparse blocks waste ~4x
    throughput vs dense 128x128 tiles and the block gather costs more than
    the 2x FLOP saving.
  - Feature-major ("transposed") layout throughout: activations live in SBUF
    as [feature_partition, batch_free]; weights are the stationary matmul
    operand, activations stream. Host pre-transposes x and the weights, so
    the device kernel needs no transposes at all.
  - bf16 inputs/weights (host cast) with fp32 PSUM accumulation: 1 cycle/row
    on the PE (fp32 is 4) and half the HBM traffic.
  - Each layer runs as "waves" of 8 output tiles: 8 PSUM banks hold the 8
    accumulators (one accumulation group per bank — a matmul with start=True
    zeroes a whole 2KB zero-region, so groups must not share a bank), the
    k-outer loop streams merged weight k-tiles from one packed sequential
    DRAM tensor across both HWDGE queues, and ReLU+bias epilogues alternate
    between ScalarE and VectorE. fc3 runs j-outer with W3 resident so its
    epilogues/stores overlap compute. Dummy warmup matmuls during the DMA
    ramp keep the PE clock (HAM) warm; Tile's end-of-kernel barrier is
    replaced by a minimal drain (NRT's own reset protocol follows anyway).

    Measured (8 cores, max-core NEFF exec): ~78-82us, rel err 4.1e-3.
    A float32r variant (kernel(..., _dt="f32r")) gives rel err 2.7e-4 at
    ~120us if tighter accuracy is required.
"""

import numpy as np
import ml_dtypes

try:
    import concourse.bass as bass  # noqa: F401
except ImportError:
    import sys

    for _p in ("/opt/trn_rl_repo", "/root/.axon_site/_ro/trn_rl_repo"):
        if _p not in sys.path:
            sys.path.insert(0, _p)

import concourse.bacc as bacc
import concourse.bass as bass
import concourse.mybir as mybir
import concourse.tile as tile
from concourse import bass_utils

LIGHT_TAIL = True  # replace Tile's heavy end-of-kernel barrier with a minimal one
FAST_CONST = True  # route Bass-init const-AP memsets to VectorE (GpSimd is ~8x slower)

B, IN, H, OUT, BS = 2048, 1024, 2048, 1024, 32
NCORES = 8
BSH = B // NCORES  # 256 batch rows per core
P = 128
WCOLS = 1024  # streamed weight tile = [P, WCOLS] = 8 output tiles of 128

F32 = mybir.dt.float32
RELU = mybir.ActivationFunctionType.Relu
IDENT = mybir.ActivationFunctionType.Identity

# Wave schedule: (kt, n_out_tiles) per wave; weights packed in this order.
# fc1: 2 waves x 8 k-tiles; fc2: 2 waves x 16; fc3: 1 wave x 16.
NW1, NW2, NW3 = 2, 2, 1
KT1, KT2, KT3 = IN // P, H // P, H // P
WSEQ_TILES = NW1 * KT1 + NW2 * KT2 + NW3 * KT3  # 64

_CACHE = {}


def _emit(tc, DT, MMDT=None):
    """MMDT: optional matmul-operand dtype (e.g. float32r); operands are
    bitcast views, storage/DMA stay in DT."""
    nc = tc.nc
    mmcast = (lambda ap: ap.bitcast(MMDT)) if MMDT is not None else (lambda ap: ap)

    xT = nc.dram_tensor("xT", [P, KT1, BSH], DT, kind="ExternalInput").ap()
    wseq = nc.dram_tensor("wseq", [WSEQ_TILES, P, WCOLS], DT, kind="ExternalInput").ap()
    bc = nc.dram_tensor("bc", [P, 2 * H // P + OUT // P], F32, kind="ExternalInput").ap()
    outT = nc.dram_tensor("outT", [OUT // P, P, BSH], F32, kind="ExternalOutput").ap()

    from contextlib import ExitStack

    with ExitStack() as ctx:
        wp = ctx.enter_context(tc.tile_pool(name="wpool", bufs=16))
        act = ctx.enter_context(tc.tile_pool(name="act", bufs=1))
        pp = ctx.enter_context(tc.tile_pool(name="ps", bufs=1, space="PSUM"))
        iop = ctx.enter_context(tc.tile_pool(name="io", bufs=1))

        # x (host-reordered even-k-first) + biases load on the Scalar queue
        # only, so the Sync queue leads with the first weight tile; k=0's
        # x-chunks arrive while w0 streams in parallel.
        xt = iop.tile([P, KT1, BSH], DT, tag="x", name="xt")
        nc.scalar.dma_start(xt[:, 0 : KT1 // 2, :], xT[:, 0 : KT1 // 2, :])
        bs = iop.tile([P, 2 * H // P + OUT // P], F32, tag="bs", name="bs")
        nc.gpsimd.dma_start(bs[:], bc[:])
        nc.gpsimd.dma_start(xt[:, KT1 // 2 : KT1, :], xT[:, KT1 // 2 : KT1, :])
        _xperm = {k: k // 2 if k % 2 == 0 else KT1 // 2 + k // 2 for k in range(KT1)}
        xts = [xt[:, _xperm[k], :] for k in range(KT1)]
        b1s = bs[:, 0 : H // P]
        b2s = bs[:, H // P : 2 * H // P]
        b3s = bs[:, 2 * H // P :]

        # PE warmup: the first real matmul can't start until the first
        # weight tile lands (~12us), guaranteeing a cold (1.2 GHz) PE via
        # the HAM clock gate. Run dummy matmuls on zeroed SBUF during the
        # DMA ramp so the PE is at 2.4 GHz when real work arrives.
        warm_rhs = iop.tile([P, BSH], mybir.dt.bfloat16, tag="warm_rhs", name="warm_rhs")
        nc.vector.memset(warm_rhs[:], 0.0)
        warm_ps = pp.tile([P, BSH], F32, tag="ps0", name="warm_ps")
        for i in range(40):
            nc.tensor.matmul(
                warm_ps[:],
                mmcast(warm_rhs[:, 0:P]),
                mmcast(warm_rhs[:]),
                start=True,
                stop=True,
            )

        wslot = [0]  # next tile index in wseq
        # Stripe weight-tile DMAs across independent per-engine HWDGE queues
        # so one queue's slot-semaphore wait doesn't idle all 16 DMA engines.
        dmaq = [nc.sync, nc.scalar]

        wdma = [0]  # weight-DMA instruction counter (for queue striping)

        def wave(kt, rhs_tiles, bias, bias_off, func, out_dt, tag, merge=2):
            """8 out tiles [P, BSH] = func(sum_k w_k.T @ rhs_k + bias).

            merge k-tiles stream per DMA instruction: a dma_start occupies
            the issuing engine ~700ns regardless of size, so fewer+bigger
            wins — except at kernel start, where small first tiles get the
            PE going sooner (merge=1 for the very first wave).
            """
            ps = [
                pp.tile([P, BSH], F32, tag=f"ps{i}", name=f"{tag}ps{i}")
                for i in range(WCOLS // P)
            ]
            for k0 in range(0, kt, merge):
                w = wp.tile([P, merge, WCOLS], DT, tag="w", name=f"{tag}w{k0}")
                src = wseq[wslot[0] : wslot[0] + merge].rearrange("i p c -> p i c")
                dmaq[wdma[0] % 2].dma_start(w[:], src)
                wdma[0] += 1
                wslot[0] += merge
                for kk in range(merge):
                    k = k0 + kk
                    for j in range(WCOLS // P):
                        nc.tensor.matmul(
                            ps[j][:],
                            mmcast(w[:, kk, j * P : (j + 1) * P]),
                            mmcast(rhs_tiles[k]),
                            start=(k == 0),
                            stop=(k == kt - 1),
                        )
            outs = []
            for j in range(WCOLS // P):
                o = act.tile([P, BSH], out_dt, tag=f"{tag}o{j}", name=f"{tag}o{j}")
                bias_ap = bias[:, bias_off + j : bias_off + j + 1]
                # Alternate epilogues between ScalarE and VectorE so the
                # per-wave epilogue chain (which gates PSUM bank reuse by the
                # next wave) halves in length.
                if j % 2 == 0 or out_dt is mybir.dt.float32r:
                    nc.scalar.activation(o[:], ps[j][:], func, bias=bias_ap)
                elif func is RELU:
                    nc.vector.tensor_scalar(
                        o[:],
                        ps[j][:],
                        bias_ap,
                        0.0,
                        mybir.AluOpType.add,
                        mybir.AluOpType.max,
                    )
                else:
                    nc.vector.tensor_scalar_add(o[:], ps[j][:], bias_ap)
                outs.append(o[:])
            return outs

        hts = []
        for wv in range(NW1):
            hts += wave(KT1, xts, b1s, wv * 8, RELU, DT, f"l1w{wv}", merge=1 if wv == 0 else 2)
        h2s = []
        for wv in range(NW2):
            h2s += wave(KT2, hts, b2s, wv * 8, RELU, DT, f"l2w{wv}")

        # fc3 runs j-outer with all of W3 resident (prefetched while fc2
        # computes): each output tile's epilogue + store overlaps the next
        # tile's matmuls, so only the last tile's epilogue is tail latency.
        w3tiles = []
        for t in range(KT3 // 2):
            w = wp.tile([P, 2, WCOLS], DT, tag="w", name=f"l3w{t}")
            src = wseq[wslot[0] : wslot[0] + 2].rearrange("i p c -> p i c")
            dmaq[wdma[0] % 2].dma_start(w[:], src)
            wdma[0] += 1
            wslot[0] += 2
            w3tiles.append(w)
        for j in range(OUT // P):
            psj = pp.tile([P, BSH], F32, tag=f"ps{j}", name=f"l3ps{j}")
            for k in range(KT3):
                nc.tensor.matmul(
                    psj[:],
                    mmcast(w3tiles[k // 2][:, k % 2, j * P : (j + 1) * P]),
                    mmcast(h2s[k]),
                    start=(k == 0),
                    stop=(k == KT3 - 1),
                )
            o = act.tile([P, BSH], F32, tag=f"l3o{j}", name=f"l3o{j}")
            if j % 2 == 0:
                nc.scalar.activation(o[:], psj[:], IDENT, bias=b3s[:, j : j + 1])
            else:
                nc.vector.tensor_scalar_add(o[:], psj[:], b3s[:, j : j + 1])
            dmaq[j % len(dmaq)].dma_start(outT[j], o[:])


class _LightTailTileContext(tile.TileContext):
    """TileContext with a minimal end-of-kernel sequence.

    Tile's default tail (drain + full all-engine barrier + DMA/semaphore
    reset + second barrier) costs ~8-10us on HW, dominated by NRT's
    expansion of the drain-with-sem-range reset. For a single-TileContext
    kernel the correctness requirement at the end is just: all engines done
    and all output DMAs complete before the NEFF signals completion.
    """

    def _drain_and_barrier(self, tick_clock, wait_clock):
        if not hasattr(self.nc, "_tile_sem_poison_stack"):
            return super()._drain_and_barrier(tick_clock, wait_clock)
        from concourse.vector_clock import ScopedClock

        drain_inst = self.nc.sync.drain()
        wait_clock.add_sem_waits(
            drain_inst.ins, ScopedClock({None: tick_clock.global_clock})
        )
        self.nc.all_engine_barrier(sem_only=True)
        assert self.sems is not None
        popped = self.nc._tile_sem_poison_stack.pop()
        assert popped is self._sem_poison


def _build(dt_name):
    if dt_name in _CACHE:
        return _CACHE[dt_name]
    DT = {"bf16": mybir.dt.bfloat16, "f32r": mybir.dt.float32r, "f32": F32}[dt_name]
    MMDT = None

    patches = []
    if FAST_CONST:
        try:
            import concourse.bass as cbass

            # During Bass construction only, reroute GpSimd memsets (the
            # framework's const-AP init) to the much faster VectorE: they
            # gate the initial all-engine barrier.
            gps_cls = cbass.BassGpSimd

            def memset_shim(self, ap, constant):
                return self.bass.vector.memset(ap, constant)

            had = "memset" in vars(gps_cls)
            orig = vars(gps_cls).get("memset")
            gps_cls.memset = memset_shim
            patches.append((gps_cls, "memset", had, orig))
            # The barrier after const-AP init protects readers of the const
            # tiles; this kernel never reads them, so skip it.
            bar_orig = cbass.Bass.all_engine_barrier

            def bar_shim(self, *, sem_only=False):
                return None

            cbass.Bass.all_engine_barrier = bar_shim
            patches.append((cbass.Bass, "all_engine_barrier", True, bar_orig))
        except AttributeError:
            pass

    try:
        nc = bacc.Bacc(
            "TRN2",
            target_bir_lowering=False,
            debug=False,
            enable_asserts=False,
            num_devices=NCORES,
        )
    finally:
        for klass, attr, had, orig in patches:
            if had:
                setattr(klass, attr, orig)
            else:
                delattr(klass, attr)

    tc_cls = _LightTailTileContext if LIGHT_TAIL else tile.TileContext
    with tc_cls(nc) as tc:
        _emit(tc, DT, MMDT)
    nc.compile()
    _CACHE[dt_name] = nc
    return nc


def _np_dt(dt_name):
    return mybir.dt.np({"bf16": mybir.dt.bfloat16, "f32r": F32, "f32": F32}[dt_name])


def _host_prep(x, W1, b1, crow_indices, col_indices, values, b2, W3, b3, npdt):
    rb = crow_indices.shape[0] - 1
    nnz, bs, _ = values.shape
    cb = H // bs
    # Scatter BSR into dense W2 [H, H].
    blocks = np.zeros((rb, cb, bs, bs), np.float32)
    row_ids = (
        np.searchsorted(crow_indices, np.arange(nnz, dtype=np.int64), side="right") - 1
    )
    blocks[row_ids, col_indices] = values
    W2 = blocks.transpose(0, 2, 1, 3).reshape(H, H)

    # Pack the streamed weight sequence: for each layer, for each wave
    # (column-half), the k-tiles [P, WCOLS] in consumption order.
    def waves(wT, kdim, nw):  # wT [kdim, ndim] -> [nw*kt, P, WCOLS]
        kt = kdim // P
        t = wT.reshape(kt, P, nw, WCOLS).astype(npdt)
        return np.ascontiguousarray(t.transpose(2, 0, 1, 3).reshape(nw * kt, P, WCOLS))

    wseq = np.concatenate(
        [
            waves(np.ascontiguousarray(W1.T), IN, NW1),
            waves(np.ascontiguousarray(W2.T), H, NW2),
            waves(np.ascontiguousarray(W3.T), H, NW3),
        ]
    )
    bc = np.ascontiguousarray(
        np.concatenate(
            [
                b1.reshape(H // P, P).T,
                b2.reshape(H // P, P).T,
                b3.reshape(OUT // P, P).T,
            ],
            axis=1,
        ).astype(np.float32)
    )
    # x -> per-core transposed shards, [P, kt, BSH], k-chunks reordered
    # even-first so each HWDGE queue can load its half in one DMA.
    xT_all = np.ascontiguousarray(x.T.astype(npdt))  # [IN, B]
    korder = [k for k in range(KT1) if k % 2 == 0] + [
        k for k in range(KT1) if k % 2 == 1
    ]
    shards = [
        np.ascontiguousarray(
            xT_all[:, c * BSH : (c + 1) * BSH]
            .reshape(KT1, P, BSH)[korder]
            .transpose(1, 0, 2)
        )
        for c in range(NCORES)
    ]
    shared = dict(wseq=wseq, bc=bc)
    return [dict(shared, xT=shards[c]) for c in range(NCORES)]


def kernel(x, W1, b1, crow_indices, col_indices, values, b2, W3, b3, _dt="bf16"):
    nc = _build(_dt)
    in_maps = _host_prep(
        np.asarray(x, np.float32),
        np.asarray(W1, np.float32),
        np.asarray(b1, np.float32),
        np.asarray(crow_indices),
        np.asarray(col_indices),
        np.asarray(values, np.float32),
        np.asarray(b2, np.float32),
        np.asarray(W3, np.float32),
        np.asarray(b3, np.float32),
        _np_dt(_dt),
    )
    res = bass_utils.run_bass_kernel_spmd(nc, in_maps, core_ids=list(range(NCORES)))
    out = np.concatenate(
        [res.results[c]["outT"].reshape(OUT, BSH).T for c in range(NCORES)], axis=0
    )
    return np.ascontiguousarray(out.astype(np.float32))



# revision 4
# speedup vs baseline: 1.0296x; 1.0296x over previous
"""Trainium2 Bass kernel for nn_BlockedMLP (dense_mlp, 8 cores).

Strategy:
  - 8-way data parallel over the batch (B=2048 -> 256 rows/core), weights
    replicated. No collectives.
  - The BSR fc2 (50% block density, 32x32 blocks) is scattered into a dense
    [H, H] matrix on the host: on the PE array a matmul costs N streamed
    columns regardless of contraction K, so 32x32 sparse blocks waste ~4x
    throughput vs dense 128x128 tiles and the block gather costs more than
    the 2x FLOP saving.
  - Feature-major ("transposed") layout throughout: activations live in SBUF
    as [feature_partition, batch_free]; weights are the stationary matmul
    operand, activations stream. Host pre-transposes x and the weights, so
    the device kernel needs no transposes at all.
  - bf16 inputs/weights (host cast) with fp32 PSUM accumulation: 1 cycle/row
    on the PE (fp32 is 4) and half the HBM traffic.
  - v2 schedule: every weight tile gets a DEDICATED SBUF slot (the whole
    16.8 MB stream fits: 128 KB/partition of the 208 KB budget), so all
    weight DMAs issue unconditionally at kernel start and the two HWDGE
    queues stream flat-out with zero slot-reuse waits (v1 lost ~2.7 us to a
    16-slot ring stall at the fc1->fc2 boundary, plus a HAM down-clock it
    triggered).  fc1/fc2 run as 4-j-tile waves alternating between two PSUM
    bank groups, so a wave's matmuls never wait on the previous wave's
    epilogues.  Epilogues run on Vector/GpSimd (alternating), keeping
    Scalar/Sync free for DMA issue and dropping the Scalar ACT-table load.
    The weight stream leads with small (128 KB) tiles so the first real
    matmul starts ~5 us earlier; PE warmup matmuls cover the DMA-queue
    ramp to keep the HAM clock ramping toward 2.4 GHz.  fc3 runs j-outer
    with per-j epilogue+store so only the last tile's epilogue is tail.
"""

import numpy as np
import ml_dtypes

try:
    import concourse.bass as bass  # noqa: F401
except ImportError:
    import sys

    for _p in ("/opt/trn_rl_repo", "/root/.axon_site/_ro/trn_rl_repo"):
        if _p not in sys.path:
            sys.path.insert(0, _p)

import concourse.bacc as bacc
import concourse.bass as bass
import concourse.mybir as mybir
import concourse.tile as tile
from concourse import bass_utils

LIGHT_TAIL = True  # replace Tile's heavy end-of-kernel barrier with a minimal one
FAST_CONST = True  # route Bass-init const-AP memsets to VectorE (GpSimd is ~8x slower)

B, IN, H, OUT, BS = 2048, 1024, 2048, 1024, 32
NCORES = 8
BSH = B // NCORES  # 256 batch rows per core
P = 128

F32 = mybir.dt.float32
ADD = mybir.AluOpType.add
MAX = mybir.AluOpType.max

KT1, KT2, KT3 = IN // P, H // P, H // P  # 8, 16, 16
W1J, W2J, W3J = H // P, H // P, OUT // P  # 16, 16, 8 j-tiles total
WAVE_J = 4  # j-tiles per wave for fc1/fc2 (PSUM bank-group double buffering)
NW1, NW2 = W1J // WAVE_J, W2J // WAVE_J  # 4, 4

# Weight DMA plan: (layer, wave, k0, merge, width_cols). Stream order ==
# consumption order; DMAs alternate between the two HWDGE queues.
def _dma_plan():
    plan = []
    # fc1 wave0 leads with small tiles so the first matmul starts early.
    for wv in range(NW1):
        merges = [1, 1, 2, 2, 2] if wv == 0 else [4, 4]
        k0 = 0
        for m in merges:
            plan.append(("l1", wv, k0, m, WAVE_J * P))
            k0 += m
        assert k0 == KT1
    for wv in range(NW2):
        k0 = 0
        for m in [4, 4, 4, 4]:
            plan.append(("l2", wv, k0, m, WAVE_J * P))
            k0 += m
        assert k0 == KT2
    k0 = 0
    for m in [2] * 8:
        plan.append(("l3", 0, k0, m, W3J * P))
        k0 += m
    assert k0 == KT3
    return plan


DMA_PLAN = _dma_plan()
WSEQ_COLS = sum(m * w for (_, _, _, m, w) in DMA_PLAN)  # 65536 (128 KB/partition)

_CACHE = {}


def _emit(tc, DT):
    nc = tc.nc

    xT = nc.dram_tensor("xT", [P, KT1, BSH], DT, kind="ExternalInput").ap()
    wseq = nc.dram_tensor("wseq", [P, WSEQ_COLS], DT, kind="ExternalInput").ap()
    bc = nc.dram_tensor("bc", [P, W1J + W2J + W3J], F32, kind="ExternalInput").ap()
    outT = nc.dram_tensor("outT", [W3J, P, BSH], F32, kind="ExternalOutput").ap()

    from contextlib import ExitStack

    with ExitStack() as ctx:
        wp = ctx.enter_context(tc.tile_pool(name="wpool", bufs=1))
        act = ctx.enter_context(tc.tile_pool(name="act", bufs=1))
        pp = ctx.enter_context(tc.tile_pool(name="ps", bufs=1, space="PSUM"))
        iop = ctx.enter_context(tc.tile_pool(name="io", bufs=1))

        # x in 4 chunks on the GpSimd SWDGE queue (keeps both HWDGE queues
        # free for the weight stream); first chunk small so fc1 k=0 is ready
        # as soon as the first weight tile lands.
        xt = iop.tile([P, KT1, BSH], DT, tag="x", name="xt")
        for lo, hi in ((0, 1), (1, 2), (2, 4), (4, KT1)):
            nc.gpsimd.dma_start(xt[:, lo:hi, :], xT[:, lo:hi, :])
        xts = [xt[:, k, :] for k in range(KT1)]
        bs = iop.tile([P, W1J + W2J + W3J], F32, tag="bs", name="bs")
        nc.gpsimd.dma_start(bs[:], bc[:])
        b1c = lambda j: bs[:, j : j + 1]
        b2c = lambda j: bs[:, W1J + j : W1J + j + 1]
        b3c = lambda j: bs[:, W1J + W2J + j : W1J + W2J + j + 1]

        # All weight DMAs issue up front into dedicated slots: the HWDGE
        # queues then stream the full 16.8 MB back-to-back with no waits.
        dmaq = [nc.sync, nc.scalar]
        wslice = {}  # (layer, wave, k) -> (tile, col_base)
        off = 0
        for d, (layer, wv, k0, merge, width) in enumerate(DMA_PLAN):
            w = wp.tile([P, merge * width], DT, tag=f"w{d}", name=f"w_{layer}x{wv}k{k0}")
            dmaq[d % 2].dma_start(w[:], wseq[:, off : off + merge * width])
            off += merge * width
            for kk in range(merge):
                wslice[(layer, wv, k0 + kk)] = (w, kk * width)

        # PE warmup: real matmuls can't start until the first weight tile
        # lands (~9.5us: DMA queue ramp); dummy matmuls on zeroed SBUF keep
        # the HAM clock ramping toward 2.4 GHz meanwhile.
        warm_rhs = iop.tile([P, BSH], mybir.dt.bfloat16, tag="warm_rhs", name="warm_rhs")
        nc.vector.memset(warm_rhs[:], 0.0)
        warm_ps = pp.tile([P, BSH], F32, tag="pA0", name="warm_ps")
        # 22 warmups: ~14 at the 1.2 GHz ramp clock + ~8 at 2.4 GHz put the
        # first real matmul at ~10.3us, matching the DMA delivery curve
        # (~310-390 GB/s from ~8.5us) so the weight stream never starves the
        # PE (a >1.5us PE idle triggers a HAM down-clock to half speed).
        for i in range(22):
            nc.tensor.matmul(
                warm_ps[:], warm_rhs[:, 0:P], warm_rhs[:], start=True, stop=True
            )

        # All epilogues on Vector (GpSimd can't read PSUM; Scalar would need
        # the ACT-table load and is busy issuing weight DMAs). With PSUM
        # bank-group double buffering the epilogue latency never gates the
        # matmul stream, and Vector is otherwise idle (~19us work total).
        def epilogue(o, ps_ap, bias_ap, relu):
            if relu:
                nc.vector.tensor_scalar(o, ps_ap, bias_ap, 0.0, ADD, MAX)
            else:
                nc.vector.tensor_scalar_add(o, ps_ap, bias_ap)

        bank = {
            "A": ["pA0", "pA1", "pA2", "pA3"],
            "B": ["pB0", "pB1", "pB2", "pB3"],
        }

        def wave(layer, wv, kt, rhs, bias_col, out_dt, grp):
            ps = [
                pp.tile([P, BSH], F32, tag=bank[grp][i], name=f"{layer}w{wv}ps{i}")
                for i in range(WAVE_J)
            ]
            for k in range(kt):
                w, base = wslice[(layer, wv, k)]
                for j in range(WAVE_J):
                    nc.tensor.matmul(
                        ps[j][:],
                        w[:, base + j * P : base + (j + 1) * P],
                        rhs[k],
                        start=(k == 0),
                        stop=(k == kt - 1),
                    )
            outs = []
            for j in range(WAVE_J):
                o = act.tile(
                    [P, BSH], out_dt, tag=f"{layer}w{wv}o{j}", name=f"{layer}w{wv}o{j}"
                )
                epilogue(o[:], ps[j][:], bias_col(wv * WAVE_J + j), True)
                outs.append(o[:])
            return outs

        hts = []
        for wv in range(NW1):
            hts += wave("l1", wv, KT1, xts, b1c, DT, "AB"[wv % 2])
        h2s = []
        for wv in range(NW2):
            h2s += wave("l2", wv, KT2, hts, b2c, DT, "AB"[wv % 2])

        # fc3 j-outer: each output tile's epilogue + store overlaps the next
        # tile's matmuls; only the last tile's epilogue+store is tail latency.
        tags8 = bank["A"] + bank["B"]
        for j in range(W3J):
            psj = pp.tile([P, BSH], F32, tag=tags8[j], name=f"l3ps{j}")
            for k in range(KT3):
                w, base = wslice[("l3", 0, k)]
                nc.tensor.matmul(
                    psj[:],
                    w[:, base + j * P : base + (j + 1) * P],
                    h2s[k],
                    start=(k == 0),
                    stop=(k == KT3 - 1),
                )
            o = act.tile([P, BSH], F32, tag=f"l3o{j}", name=f"l3o{j}")
            epilogue(o[:], psj[:], b3c(j), False)
            dmaq[j % 2].dma_start(outT[j], o[:])


class _LightTailTileContext(tile.TileContext):
    """TileContext with a minimal end-of-kernel sequence.

    Tile's default tail (drain + full all-engine barrier + DMA/semaphore
    reset + second barrier) costs ~8-10us on HW. For a single-TileContext
    kernel the correctness requirement at the end is just: all engines done
    and all output DMAs complete before the NEFF signals completion (the
    walrus-generated per-engine teardown follows anyway).
    """

    def _drain_and_barrier(self, tick_clock, wait_clock):
        if not hasattr(self.nc, "_tile_sem_poison_stack"):
            return super()._drain_and_barrier(tick_clock, wait_clock)
        from concourse.vector_clock import ScopedClock

        drain_inst = self.nc.sync.drain()
        wait_clock.add_sem_waits(
            drain_inst.ins, ScopedClock({None: tick_clock.global_clock})
        )
        self.nc.all_engine_barrier(sem_only=True)
        assert self.sems is not None
        popped = self.nc._tile_sem_poison_stack.pop()
        assert popped is self._sem_poison

def _build(dt_name):
    if dt_name in _CACHE:
        return _CACHE[dt_name]
    DT = {"bf16": mybir.dt.bfloat16, "f32": F32}[dt_name]

    patches = []
    if FAST_CONST:
        try:
            import concourse.bass as cbass

            # During Bass construction only, reroute GpSimd memsets (the
            # framework's const-AP init) to the much faster VectorE: they
            # gate the initial all-engine barrier.
            gps_cls = cbass.BassGpSimd

            def memset_shim(self, ap, constant):
                return self.bass.vector.memset(ap, constant)

            had = "memset" in vars(gps_cls)
            orig = vars(gps_cls).get("memset")
            gps_cls.memset = memset_shim
            patches.append((gps_cls, "memset", had, orig))
            # The barrier after const-AP init protects readers of the const
            # tiles; this kernel never reads them, so skip it.
            bar_orig = cbass.Bass.all_engine_barrier

            def bar_shim(self, *, sem_only=False):
                return None

            cbass.Bass.all_engine_barrier = bar_shim
            patches.append((cbass.Bass, "all_engine_barrier", True, bar_orig))
        except AttributeError:
            pass

    try:
        nc = bacc.Bacc(
            "TRN2",
            target_bir_lowering=False,
            debug=False,
            enable_asserts=False,
            num_devices=NCORES,
        )
    finally:
        for klass, attr, had, orig in patches:
            if had:
                setattr(klass, attr, orig)
            else:
                delattr(klass, attr)

    tc_cls = _LightTailTileContext if LIGHT_TAIL else tile.TileContext
    with tc_cls(nc) as tc:
        _emit(tc, DT)
    nc.compile()
    _CACHE[dt_name] = nc
    return nc


def _np_dt(dt_name):
    return mybir.dt.np({"bf16": mybir.dt.bfloat16, "f32": F32}[dt_name])


def _host_prep(x, W1, b1, crow_indices, col_indices, values, b2, W3, b3, npdt):
    rb = crow_indices.shape[0] - 1
    nnz, bs, _ = values.shape
    cb = H // bs
    # Scatter BSR into dense W2 [H, H].
    blocks = np.zeros((rb, cb, bs, bs), np.float32)
    row_ids = (
        np.searchsorted(crow_indices, np.arange(nnz, dtype=np.int64), side="right") - 1
    )
    blocks[row_ids, col_indices] = values
    W2 = blocks.transpose(0, 2, 1, 3).reshape(H, H)

    WT = {
        "l1": np.ascontiguousarray(W1.T).astype(npdt),  # [IN, H]
        "l2": np.ascontiguousarray(W2.T).astype(npdt),  # [H, H]
        "l3": np.ascontiguousarray(W3.T).astype(npdt),  # [H, OUT]
    }
    # Pack the streamed weight sequence: one contiguous [P, merge*width]
    # block per DMA instruction, in consumption order.
    blocks_out = []
    for layer, wv, k0, merge, width in DMA_PLAN:
        w = WT[layer]
        jbase = wv * width
        blocks_out.append(
            np.concatenate(
                [
                    w[(k0 + kk) * P : (k0 + kk + 1) * P, jbase : jbase + width]
                    for kk in range(merge)
                ],
                axis=1,
            )
        )
    wseq = np.ascontiguousarray(np.concatenate(blocks_out, axis=1))
    assert wseq.shape == (P, WSEQ_COLS)

    bc = np.ascontiguousarray(
        np.concatenate(
            [
                b1.reshape(W1J, P).T,
                b2.reshape(W2J, P).T,
                b3.reshape(W3J, P).T,
            ],
            axis=1,
        ).astype(np.float32)
    )
    # x -> per-core transposed shards [P, KT1, BSH] in natural k order.
    xT_all = np.ascontiguousarray(x.T.astype(npdt))  # [IN, B]
    shards = [
        np.ascontiguousarray(
            xT_all[:, c * BSH : (c + 1) * BSH].reshape(KT1, P, BSH).transpose(1, 0, 2)
        )
        for c in range(NCORES)
    ]
    shared = dict(wseq=wseq, bc=bc)
    return [dict(shared, xT=shards[c]) for c in range(NCORES)]


def kernel(x, W1, b1, crow_indices, col_indices, values, b2, W3, b3, _dt="bf16"):
    nc = _build(_dt)
    in_maps = _host_prep(
        np.asarray(x, np.float32),
        np.asarray(W1, np.float32),
        np.asarray(b1, np.float32),
        np.asarray(crow_indices),
        np.asarray(col_indices),
        np.asarray(values, np.float32),
        np.asarray(b2, np.float32),
        np.asarray(W3, np.float32),
        np.asarray(b3, np.float32),
        _np_dt(_dt),
    )
    res = bass_utils.run_bass_kernel_spmd(nc, in_maps, core_ids=list(range(NCORES)))
    out = np.concatenate(
        [res.results[c]["outT"].reshape(OUT, BSH).T for c in range(NCORES)], axis=0
    )
    return np.ascontiguousarray(out.astype(np.float32))


# revision 6
# speedup vs baseline: 1.0474x; 1.0173x over previous
"""Trainium2 Bass kernel for nn_BlockedMLP (dense_mlp, 8 cores).

Strategy:
  - 8-way data parallel over the batch (B=2048 -> 256 rows/core), weights
    replicated. No collectives.
  - The BSR fc2 (50% block density, 32x32 blocks) is scattered into a dense
    [H, H] matrix on the host: on the PE array a matmul costs N streamed
    columns regardless of contraction K, so 32x32 sparse blocks waste ~4x
    throughput vs dense 128x128 tiles and the block gather costs more than
    the 2x FLOP saving.
  - Feature-major ("transposed") layout throughout: activations live in SBUF
    as [feature_partition, batch_free]; weights are the stationary matmul
    operand, activations stream. Host pre-transposes x and the weights, so
    the device kernel needs no transposes at all.
  - bf16 inputs/weights (host cast) with fp32 PSUM accumulation: 1 cycle/row
    on the PE (fp32 is 4) and half the HBM traffic.
  - v2 schedule: every weight tile gets a DEDICATED SBUF slot (the whole
    16.8 MB stream fits: 128 KB/partition of the 208 KB budget), so all
    weight DMAs issue unconditionally at kernel start and the two HWDGE
    queues stream flat-out with zero slot-reuse waits (v1 lost ~2.7 us to a
    16-slot ring stall at the fc1->fc2 boundary, plus a HAM down-clock it
    triggered).  fc1/fc2 run as 4-j-tile waves alternating between two PSUM
    bank groups, so a wave's matmuls never wait on the previous wave's
    epilogues.  Epilogues run on Vector/GpSimd (alternating), keeping
    Scalar/Sync free for DMA issue and dropping the Scalar ACT-table load.
    The weight stream leads with small (128 KB) tiles so the first real
    matmul starts ~5 us earlier; PE warmup matmuls cover the DMA-queue
    ramp to keep the HAM clock ramping toward 2.4 GHz.  fc3 runs j-outer
    with per-j epilogue+store so only the last tile's epilogue is tail.
"""

import numpy as np
import ml_dtypes

try:
    import concourse.bass as bass  # noqa: F401
except ImportError:
    import sys

    for _p in ("/opt/trn_rl_repo", "/root/.axon_site/_ro/trn_rl_repo"):
        if _p not in sys.path:
            sys.path.insert(0, _p)

import concourse.bacc as bacc
import concourse.bass as bass
import concourse.mybir as mybir
import concourse.tile as tile
from concourse import bass_utils

LIGHT_TAIL = True  # replace Tile's heavy end-of-kernel barrier with a minimal one
FAST_CONST = True  # route Bass-init const-AP memsets to VectorE (GpSimd is ~8x slower)

B, IN, H, OUT, BS = 2048, 1024, 2048, 1024, 32
NCORES = 8
BSH = B // NCORES  # 256 batch rows per core
P = 128

F32 = mybir.dt.float32
ADD = mybir.AluOpType.add
MAX = mybir.AluOpType.max

KT1, KT2, KT3 = IN // P, H // P, H // P  # 8, 16, 16
W1J, W2J, W3J = H // P, H // P, OUT // P  # 16, 16, 8 j-tiles total
WAVE_J = 4  # j-tiles per wave for fc1/fc2 (PSUM bank-group double buffering)
NW1, NW2 = W1J // WAVE_J, W2J // WAVE_J  # 4, 4

# Weight DMA plan: (layer, wave, k0, merge, width_cols). Stream order ==
# consumption order; DMAs alternate between the two HWDGE queues.
def _dma_plan():
    plan = []
    # fc1 leads with small tiles: the first matmul starts early, and the
    # fine per-DMA granularity means a k-tile is usable as soon as its own
    # 128-256KB lands (a merge-4 tile would gate k0 on all 512KB arriving,
    # which stalls the PE during the DMA ramp).
    for wv in range(NW1):
        merges = [1, 1, 2, 2, 2] if wv == 0 else [2, 2, 2, 2]
        k0 = 0
        for m in merges:
            plan.append(("l1", wv, k0, m, WAVE_J * P))
            k0 += m
        assert k0 == KT1
    for wv in range(NW2):
        k0 = 0
        for m in [4, 4, 4, 4]:
            plan.append(("l2", wv, k0, m, WAVE_J * P))
            k0 += m
        assert k0 == KT2
    k0 = 0
    for m in [2] * 8:
        plan.append(("l3", 0, k0, m, W3J * P))
        k0 += m
    assert k0 == KT3
    return plan


DMA_PLAN = _dma_plan()
WSEQ_COLS = sum(m * w for (_, _, _, m, w) in DMA_PLAN)  # 65536 (128 KB/partition)

_CACHE = {}


def _emit(tc, DT):
    nc = tc.nc

    xT = nc.dram_tensor("xT", [P, KT1, BSH], DT, kind="ExternalInput").ap()
    wseq = nc.dram_tensor("wseq", [P, WSEQ_COLS], DT, kind="ExternalInput").ap()
    bc = nc.dram_tensor("bc", [P, W1J + W2J + W3J], F32, kind="ExternalInput").ap()
    outT = nc.dram_tensor("outT", [W3J, P, BSH], F32, kind="ExternalOutput").ap()

    from contextlib import ExitStack

    with ExitStack() as ctx:
        wp = ctx.enter_context(tc.tile_pool(name="wpool", bufs=1))
        act = ctx.enter_context(tc.tile_pool(name="act", bufs=1))
        pp = ctx.enter_context(tc.tile_pool(name="ps", bufs=1, space="PSUM"))
        iop = ctx.enter_context(tc.tile_pool(name="io", bufs=1))

        # x in 4 chunks on the GpSimd SWDGE queue (keeps both HWDGE queues
        # free for the weight stream); first chunk small so fc1 k=0 is ready
        # as soon as the first weight tile lands.
        xt = iop.tile([P, KT1, BSH], DT, tag="x", name="xt")
        for lo, hi in ((0, 1), (1, 2), (2, 4), (4, KT1)):
            nc.gpsimd.dma_start(xt[:, lo:hi, :], xT[:, lo:hi, :])
        xts = [xt[:, k, :] for k in range(KT1)]
        bs = iop.tile([P, W1J + W2J + W3J], F32, tag="bs", name="bs")
        nc.gpsimd.dma_start(bs[:], bc[:])
        b1c = lambda j: bs[:, j : j + 1]
        b2c = lambda j: bs[:, W1J + j : W1J + j + 1]
        b3c = lambda j: bs[:, W1J + W2J + j : W1J + W2J + j + 1]

        # All weight DMAs issue up front into dedicated slots: the HWDGE
        # queues then stream the full 16.8 MB back-to-back with no waits.
        dmaq = [nc.sync, nc.scalar]
        wslice = {}  # (layer, wave, k) -> (tile, col_base)
        off = 0
        for d, (layer, wv, k0, merge, width) in enumerate(DMA_PLAN):
            w = wp.tile([P, merge * width], DT, tag=f"w{d}", name=f"w_{layer}x{wv}k{k0}")
            dmaq[d % 2].dma_start(w[:], wseq[:, off : off + merge * width])
            off += merge * width
            for kk in range(merge):
                wslice[(layer, wv, k0 + kk)] = (w, kk * width)

        # PE warmup: real matmuls can't start until the first weight tile
        # lands (~9.5us: DMA queue ramp); dummy matmuls on zeroed SBUF keep
        # the HAM clock ramping toward 2.4 GHz meanwhile.
        warm_rhs = iop.tile([P, BSH], mybir.dt.bfloat16, tag="warm_rhs", name="warm_rhs")
        nc.vector.memset(warm_rhs[:], 0.0)
        warm_ps = pp.tile([P, BSH], F32, tag="pA0", name="warm_ps")
        # ~18 warmups put the first real matmul at ~10us, matching the DMA
        # delivery curve (~200-390 GB/s from ~8.5us) so the weight stream
        # rarely starves the PE (a >1.5us PE idle triggers a HAM down-clock
        # to half speed).
        for i in range(18):
            nc.tensor.matmul(
                warm_ps[:], warm_rhs[:, 0:P], warm_rhs[:], start=True, stop=True
            )

        # All epilogues on Vector (GpSimd can't read PSUM; Scalar would need
        # the ACT-table load and is busy issuing weight DMAs). With PSUM
        # bank-group double buffering the epilogue latency never gates the
        # matmul stream, and Vector is otherwise idle (~19us work total).
        def epilogue(o, ps_ap, bias_ap, relu):
            if relu:
                nc.vector.tensor_scalar(o, ps_ap, bias_ap, 0.0, ADD, MAX)
            else:
                nc.vector.tensor_scalar_add(o, ps_ap, bias_ap)

        bank = {
            "A": ["pA0", "pA1", "pA2", "pA3"],
            "B": ["pB0", "pB1", "pB2", "pB3"],
        }

        def wave(layer, wv, kt, rhs, bias_col, out_dt, grp):
            ps = [
                pp.tile([P, BSH], F32, tag=bank[grp][i], name=f"{layer}w{wv}ps{i}")
                for i in range(WAVE_J)
            ]
            for k in range(kt):
                w, base = wslice[(layer, wv, k)]
                for j in range(WAVE_J):
                    nc.tensor.matmul(
                        ps[j][:],
                        w[:, base + j * P : base + (j + 1) * P],
                        rhs[k],
                        start=(k == 0),
                        stop=(k == kt - 1),
                    )
            outs = []
            for j in range(WAVE_J):
                o = act.tile(
                    [P, BSH], out_dt, tag=f"{layer}w{wv}o{j}", name=f"{layer}w{wv}o{j}"
                )
                epilogue(o[:], ps[j][:], bias_col(wv * WAVE_J + j), True)
                outs.append(o[:])
            return outs

        hts = []
        for wv in range(NW1):
            hts += wave("l1", wv, KT1, xts, b1c, DT, "AB"[wv % 2])
        h2s = []
        for wv in range(NW2):
            h2s += wave("l2", wv, KT2, hts, b2c, DT, "AB"[wv % 2])

        # fc3 j-outer: each output tile's epilogue + store overlaps the next
        # tile's matmuls; only the last tile's epilogue+store is tail latency.
        tags8 = bank["A"] + bank["B"]
        for j in range(W3J):
            psj = pp.tile([P, BSH], F32, tag=tags8[j], name=f"l3ps{j}")
            for k in range(KT3):
                w, base = wslice[("l3", 0, k)]
                nc.tensor.matmul(
                    psj[:],
                    w[:, base + j * P : base + (j + 1) * P],
                    h2s[k],
                    start=(k == 0),
                    stop=(k == KT3 - 1),
                )
            o = act.tile([P, BSH], F32, tag=f"l3o{j}", name=f"l3o{j}")
            epilogue(o[:], psj[:], b3c(j), False)
            dmaq[j % 2].dma_start(outT[j], o[:])


class _LightTailTileContext(tile.TileContext):
    """TileContext with a minimal end-of-kernel sequence.

    Tile's default tail (drain + full all-engine barrier + DMA/semaphore
    reset + second barrier) costs ~8-10us on HW. For a single-TileContext
    kernel the correctness requirement at the end is just: all engines done
    and all output DMAs complete before the NEFF signals completion (the
    walrus-generated per-engine teardown follows anyway).
    """

    def _drain_and_barrier(self, tick_clock, wait_clock):
        if not hasattr(self.nc, "_tile_sem_poison_stack"):
            return super()._drain_and_barrier(tick_clock, wait_clock)
        from concourse.vector_clock import ScopedClock

        drain_inst = self.nc.sync.drain()
        wait_clock.add_sem_waits(
            drain_inst.ins, ScopedClock({None: tick_clock.global_clock})
        )
        self.nc.all_engine_barrier(sem_only=True)
        assert self.sems is not None
        popped = self.nc._tile_sem_poison_stack.pop()
        assert popped is self._sem_poison

def _build(dt_name):
    if dt_name in _CACHE:
        return _CACHE[dt_name]
    DT = {"bf16": mybir.dt.bfloat16, "f32": F32}[dt_name]

    patches = []
    if FAST_CONST:
        try:
            import concourse.bass as cbass

            # During Bass construction only, reroute GpSimd memsets (the
            # framework's const-AP init) to the much faster VectorE: they
            # gate the initial all-engine barrier.
            gps_cls = cbass.BassGpSimd

            def memset_shim(self, ap, constant):
                return self.bass.vector.memset(ap, constant)

            had = "memset" in vars(gps_cls)
            orig = vars(gps_cls).get("memset")
            gps_cls.memset = memset_shim
            patches.append((gps_cls, "memset", had, orig))
            # The barrier after const-AP init protects readers of the const
            # tiles; this kernel never reads them, so skip it.
            bar_orig = cbass.Bass.all_engine_barrier

            def bar_shim(self, *, sem_only=False):
                return None

            cbass.Bass.all_engine_barrier = bar_shim
            patches.append((cbass.Bass, "all_engine_barrier", True, bar_orig))
        except AttributeError:
            pass

    try:
        nc = bacc.Bacc(
            "TRN2",
            target_bir_lowering=False,
            debug=False,
            enable_asserts=False,
            num_devices=NCORES,
        )
    finally:
        for klass, attr, had, orig in patches:
            if had:
                setattr(klass, attr, orig)
            else:
                delattr(klass, attr)

    tc_cls = _LightTailTileContext if LIGHT_TAIL else tile.TileContext
    with tc_cls(nc) as tc:
        _emit(tc, DT)
    nc.compile()
    _CACHE[dt_name] = nc
    return nc


def _np_dt(dt_name):
    return mybir.dt.np({"bf16": mybir.dt.bfloat16, "f32": F32}[dt_name])


def _host_prep(x, W1, b1, crow_indices, col_indices, values, b2, W3, b3, npdt):
    rb = crow_indices.shape[0] - 1
    nnz, bs, _ = values.shape
    cb = H // bs
    # Scatter BSR into dense W2 [H, H].
    blocks = np.zeros((rb, cb, bs, bs), np.float32)
    row_ids = (
        np.searchsorted(crow_indices, np.arange(nnz, dtype=np.int64), side="right") - 1
    )
    blocks[row_ids, col_indices] = values
    W2 = blocks.transpose(0, 2, 1, 3).reshape(H, H)

    WT = {
        "l1": np.ascontiguousarray(W1.T).astype(npdt),  # [IN, H]
        "l2": np.ascontiguousarray(W2.T).astype(npdt),  # [H, H]
        "l3": np.ascontiguousarray(W3.T).astype(npdt),  # [H, OUT]
    }
    # Pack the streamed weight sequence: one contiguous [P, merge*width]
    # block per DMA instruction, in consumption order.
    blocks_out = []
    for layer, wv, k0, merge, width in DMA_PLAN:
        w = WT[layer]
        jbase = wv * width
        blocks_out.append(
            np.concatenate(
                [
                    w[(k0 + kk) * P : (k0 + kk + 1) * P, jbase : jbase + width]
                    for kk in range(merge)
                ],
                axis=1,
            )
        )
    wseq = np.ascontiguousarray(np.concatenate(blocks_out, axis=1))
    assert wseq.shape == (P, WSEQ_COLS)

    bc = np.ascontiguousarray(
        np.concatenate(
            [
                b1.reshape(W1J, P).T,
                b2.reshape(W2J, P).T,
                b3.reshape(W3J, P).T,
            ],
            axis=1,
        ).astype(np.float32)
    )
    # x -> per-core transposed shards [P, KT1, BSH] in natural k order.
    xT_all = np.ascontiguousarray(x.T.astype(npdt))  # [IN, B]
    shards = [
        np.ascontiguousarray(
            xT_all[:, c * BSH : (c + 1) * BSH].reshape(KT1, P, BSH).transpose(1, 0, 2)
        )
        for c in range(NCORES)
    ]
    shared = dict(wseq=wseq, bc=bc)
    return [dict(shared, xT=shards[c]) for c in range(NCORES)]


def kernel(x, W1, b1, crow_indices, col_indices, values, b2, W3, b3, _dt="bf16"):
    nc = _build(_dt)
    in_maps = _host_prep(
        np.asarray(x, np.float32),
        np.asarray(W1, np.float32),
        np.asarray(b1, np.float32),
        np.asarray(crow_indices),
        np.asarray(col_indices),
        np.asarray(values, np.float32),
        np.asarray(b2, np.float32),
        np.asarray(W3, np.float32),
        np.asarray(b3, np.float32),
        _np_dt(_dt),
    )
    res = bass_utils.run_bass_kernel_spmd(nc, in_maps, core_ids=list(range(NCORES)))
    out = np.concatenate(
        [res.results[c]["outT"].reshape(OUT, BSH).T for c in range(NCORES)], axis=0
    )
    return np.ascontiguousarray(out.astype(np.float32))
